# revision 23
# baseline (speedup 1.0000x reference)
"""AlibiCausalSelfAttention on 8 Trainium2 NeuronCores — v3.

Sharding: data-parallel over batch (B=2) x head-parallel over head groups
(16 heads -> 4 groups of 4, strided: group g = {g, g+4, g+8, g+12}).
Core c handles batch c//4, head group c%4. Each core computes a partial
projection output fp16 (W_proj row-sharded); the host sums the 4 partials
per batch in fp32 and adds an effective bias (b_proj + bv @ W_proj — the
v-bias commutes through softmax exactly).

v3 structural changes vs v2 (175us):
  - All bulk DMAs use host-prepacked per-partition-contiguous layouts
    (~128 descriptors/MB instead of ~1000), cutting DGE trigger time and
    letting the input stream start right after the ~7us framework preamble.
  - Bias matmuls eliminated: v-bias folded into the host-side output bias
    (exact); k-bias dropped (q.bk + bq.bk are per-row constants — exact
    softmax invariance); q-bias applied during the PSUM->SBUF evacuation
    via tensor_scalar_add (free).
  - Warmup junk matmuls read an uninitialized SBUF tile (no memset dep),
    so they issue as soon as the PE queue preamble ends and keep the HAM
    clock gate warm through the input-DMA window.
  - Banded attention for the steepest head of each group (global heads
    0-3, local slot 0): only j-pairs within 512 tokens of the diagonal are
    computed (dropped softmax terms < e^-18 relative — far below fp16
    resolution). Identical program on all cores; ~9% less QK/PV/exp work.
  - v-evacuation merged to one strided copy per t-chunk; softmax
    normalization reads the denominator rows straight out of PSUM
    (reciprocal_approx_fast), dropping the staging copy.
  - vaug ones-columns memset on the idle GPSIMD engine.
"""

import sys

if "/opt/trn_rl_repo" not in sys.path:
    sys.path.insert(0, "/opt/trn_rl_repo")

import numpy as np

T = 2048
C = 1024
H = 16
D = 64
HL = 4          # heads per core
HD = HL * D     # 256 local head dims
IW = 512        # i-tile width
NT16 = T // 128
NW = 16         # warmup junk matmuls

# local-slot bands (tokens below diagonal, multiple of 256; None = full).
# Slot 0 carries global heads 0..3 (slopes 1/2..1/16): need >= 18/slope =
# 288 tokens; 512 is safely generous. Slots 1-3 carry heads 4-15 (full).
BANDS = [512, None, None, None]

_CACHE = {}


def _build_nc():
    import concourse.mybir as mybir
    import concourse.tile as tile
    from concourse import bacc
    from contextlib import ExitStack

    f32 = mybir.dt.float32
    fr = mybir.dt.float16
    f8 = mybir.dt.float8e4
    DR = mybir.MatmulPerfMode.DoubleRow
    Mult = mybir.AluOpType.mult
    Add = mybir.AluOpType.add
    Exp = mybir.ActivationFunctionType.Exp
    Copy = mybir.ActivationFunctionType.Copy
    Min = mybir.AluOpType.min

    def pj0(it, h):
        b = BANDS[h]
        if b is None:
            return 0
        return max(0, (it * IW - b) // 256)

    nc = bacc.Bacc("TRN2", target_bir_lowering=False, debug=False, num_devices=8)

    xq_d = nc.dram_tensor("xq", [128, 4, 8, IW], fr, kind="ExternalInput").ap()
    x8_d = nc.dram_tensor("x8", [128, 4, 4, 2, IW], f8, kind="ExternalInput").ap()
    wqk8_d = nc.dram_tensor("wqk8", [128, 4, 4, 2, 128], f8, kind="ExternalInput").ap()
    bq_d = nc.dram_tensor("bq", [128, 2], f32, kind="ExternalInput").ap()
    wv_d = nc.dram_tensor("wv", [128, 8, HD], fr, kind="ExternalInput").ap()
    wp_d = nc.dram_tensor("wp", [128, 2, C], fr, kind="ExternalInput").ap()
    aq_d = nc.dram_tensor("aq", [2 * HL, T], fr, kind="ExternalInput").ap()
    ak_d = nc.dram_tensor("ak", [2, T], fr, kind="ExternalInput").ap()
    umin_d = nc.dram_tensor("umin", [128, 128], fr, kind="ExternalInput").ap()
    out_d = nc.dram_tensor("out", [128, NT16, C], fr, kind="ExternalOutput").ap()

    with tile.TileContext(nc) as tc, ExitStack() as ctx:
        pers = ctx.enter_context(tc.tile_pool(name="pers", bufs=1))
        work = ctx.enter_context(tc.tile_pool(name="work", bufs=3))
        rpool = ctx.enter_context(tc.tile_pool(name="rpool", bufs=2))
        ps_st = ctx.enter_context(tc.tile_pool(name="ps_st", bufs=2, space="PSUM"))
        ps_y = ctx.enter_context(tc.tile_pool(name="ps_y", bufs=2, space="PSUM"))
        ps_o = ctx.enter_context(tc.tile_pool(name="ps_o", bufs=2, space="PSUM"))

        # ---- persistent tiles ----
        qaug = [pers.tile([128, T], fr, tag=f"qaug{h}", name=f"qaug{h}") for h in range(HL)]
        kaug = [pers.tile([128, T], fr, tag=f"kaug{h}", name=f"kaug{h}") for h in range(HL)]
        vaug = [pers.tile([128, HL, 128], fr, tag=f"vaug{t}", name=f"vaug{t}")
                for t in range(NT16)]
        yT = [pers.tile([128, T], fr, tag=f"yT{i}", name=f"yT{i}") for i in range(HL // 2)]
        xs_q = [pers.tile([128, 8, IW], fr, tag=f"xs{q}", name=f"xs{q}")
                for q in range(4)]
        x8_q = [pers.tile([128, 4, 2, IW], f8, tag=f"x8{q}", name=f"x8{q}")
                for q in range(4)]
        wqk8_t = pers.tile([128, 4, 4, 2, 128], f8, tag="wqk8")
        wv_t = pers.tile([128, 8, HD], fr, tag="wvt")
        wp_t = pers.tile([128, 2, C], fr, tag="wpt")
        bq_t = pers.tile([128, 2], f32, tag="bq")
        umin = pers.tile([128, 128], fr, tag="umin")
        junk = pers.tile([128, IW], fr, tag="junk")

        # ---- warmup junk matmuls: minimal deps (one gpsimd memset), keep the
        # PE/HAM warm while inputs stream in.
        nc.gpsimd.memset(junk[:], 0.0)
        for w in range(NW):
            psd = ps_st.tile([128, IW], f32, tag="st", name="psd")
            nc.tensor.matmul(psd[:], junk[:, 0:128], junk[:], start=True, stop=True)

        # ---- DMAs: per-partition-contiguous both sides; ordered so the
        # minimal prefix for early compute lands first.
        nc.sync.dma_start(wqk8_t[:], wqk8_d[:])
        nc.sync.dma_start(x8_q[0][:], x8_d[:, 0, :, :, :])
        nc.sync.dma_start(xs_q[0][:, 0:4, :], xq_d[:, 0, 0:4, :])
        nc.sync.dma_start(xs_q[0][:, 4:8, :], xq_d[:, 0, 4:8, :])
        nc.sync.dma_start(wv_t[:], wv_d[:])
        nc.sync.dma_start(x8_q[1][:], x8_d[:, 1, :, :, :])
        nc.sync.dma_start(xs_q[1][:], xq_d[:, 1, :, :])
        nc.sync.dma_start(wp_t[:], wp_d[:])
        nc.sync.dma_start(x8_q[2][:], x8_d[:, 2, :, :, :])
        nc.sync.dma_start(xs_q[2][:], xq_d[:, 2, :, :])
        nc.sync.dma_start(x8_q[3][:], x8_d[:, 3, :, :, :])
        nc.sync.dma_start(xs_q[3][:], xq_d[:, 3, :, :])
        # small constants on the scalar HWDGE queue
        nc.scalar.dma_start(bq_t[:], bq_d[:])
        nc.scalar.dma_start(umin[:], umin_d[:])
        for h in range(HL):
            nc.scalar.dma_start(qaug[h][64:66, :], aq_d[2 * h:2 * h + 2, :])
            nc.scalar.dma_start(kaug[h][64:66, :], ak_d[:, :])
        # ones columns of vaug on the idle GPSIMD engine
        for t in range(NT16):
            nc.gpsimd.memset(vaug[t][:, :, 0:64], 1.0)

        # ---------------- phase 1 helpers ----------------
        def qk_tile(cc, it):
            ps = ps_o.tile([128, IW], f32, tag="o", name="qkps")
            for kp in range(4):
                nc.tensor.matmul(
                    ps[:], wqk8_t[:, cc, kp, :, :], x8_q[it][:, kp, :, :],
                    start=(kp == 0), stop=(kp == 3), perf_mode=DR)
            for half in range(2):
                h = (cc % 2) * 2 + half
                src = ps[half * 64:(half + 1) * 64, :]
                if cc < 2:
                    nc.vector.tensor_scalar(
                        qaug[h][0:64, it * IW:(it + 1) * IW], src,
                        1.0 / 256.0, bq_t[half * 64:(half + 1) * 64, cc:cc + 1],
                        Mult, Add)
                else:
                    nc.vector.tensor_scalar(
                        kaug[h][0:64, it * IW:(it + 1) * IW], src,
                        1.0 / 32.0, None, Mult)

        def v_tile(t16):
            ps = ps_o.tile([128, HL, 64], f32, tag="o", name="vps")
            qq, sub = t16 // 4, (t16 % 4) * 128
            for k in range(8):
                nc.tensor.matmul(
                    ps[:], xs_q[qq][:, k, sub:sub + 128], wv_t[:, k, :],
                    start=(k == 0), stop=(k == 7))
            nc.scalar.activation(vaug[t16][:, :, 64:128], ps[:], Copy)

        # ---------------- attention for one (i-tile, head) ----------------
        def attn(it, h, fillers=(), carry=None):
            fillers = list(fillers)
            i0 = it * IW
            njc = i0 // 128 + IW // 128
            npair = njc // 2
            p0 = pj0(it, h)
            yacc = ps_y.tile([128, IW], f32, tag="yacc", name="yacc")
            pend = None

            def emit_pv(p, c0a, c0b, Wa, Wb, pj):
                nc.tensor.matmul(
                    yacc[:, c0a:IW], vaug[2 * pj][:, h, :], p[:, 0:Wa],
                    start=(pj == p0), stop=False)
                nc.tensor.matmul(
                    yacc[:, c0b:IW], vaug[2 * pj + 1][:, h, :], p[:, Wa:Wa + Wb],
                    start=False, stop=(pj == npair - 1))

            for pj in range(p0, npair):
                j0a = (2 * pj) * 128
                j0b = j0a + 128
                c0a = max(0, j0a - i0)
                c0b = max(0, j0b - i0)
                Wa = IW - c0a
                Wb = IW - c0b
                st2 = ps_st.tile([128, 2 * IW], f32, tag="st", name="st")
                nc.tensor.matmul(
                    st2[:, 0:Wa],
                    kaug[h][0:66, j0a:j0a + 128],
                    qaug[h][0:66, i0 + c0a:i0 + IW],
                    start=True, stop=True)
                nc.tensor.matmul(
                    st2[:, Wa:Wa + Wb],
                    kaug[h][0:66, j0b:j0b + 128],
                    qaug[h][0:66, i0 + c0b:i0 + IW],
                    start=True, stop=True)
                if pend is not None:
                    emit_pv(*pend)
                    pend = None
                    if fillers:
                        fillers.pop(0)()
                elif carry is not None:
                    carry()
                    carry = None
                p = work.tile([128, 2 * IW], fr, tag="p", name="p")
                nc.scalar.activation(p[:, 0:Wa + Wb], st2[:, 0:Wa + Wb], Exp)
                if j0a >= i0:
                    nc.vector.tensor_tensor(p[:, 0:128], p[:, 0:128], umin[:], Min)
                    nc.vector.tensor_tensor(
                        p[:, Wa:Wa + 128], p[:, Wa:Wa + 128], umin[:], Min)
                pend = (p, c0a, c0b, Wa, Wb, pj)
            for f in fillers:
                f()
            last = pend

            def finish():
                if carry is not None:
                    carry()
                emit_pv(*last)
                # denominator rows 0:64 of yacc (vaug ones-cols broadcast);
                # read directly from PSUM at partition offset 0
                rec = rpool.tile([64, IW], f32, tag="rec", name="rec")
                nc.vector.reciprocal_approx_fast(rec[:], yacc[0:64, :])
                nc.vector.tensor_mul(
                    yT[h // 2][(h % 2) * 64:(h % 2) * 64 + 64, i0:i0 + IW],
                    yacc[64:128, :], rec[:])

            return finish

        def outproj_units(tp):
            ot = work.tile([128, 2, C], fr, tag="ot", name="ot")

            def unit(half, e2):
                def go():
                    t16 = 2 * tp + half
                    ps = ps_o.tile([128, 512], f32, tag="o", name="ops")
                    for kk in range(2):
                        nc.tensor.matmul(
                            ps[:],
                            yT[kk][:, t16 * 128:(t16 + 1) * 128],
                            wp_t[:, kk, e2 * 512:(e2 + 1) * 512],
                            start=(kk == 0), stop=(kk == 1))
                    nc.vector.tensor_copy(
                        ot[:, half, e2 * 512:(e2 + 1) * 512], ps[:])
                    if e2 == 1:
                        nc.sync.dma_start(
                            out_d[:, 2 * tp + half, :], ot[:, half, :])
                return go

            return [unit(0, 0), unit(0, 1), unit(1, 0), unit(1, 1)]

        def outproj_tp(tp):
            for f in outproj_units(tp):
                f()


        # ---------------- program order ----------------
        c = None
        for it in range(4):
            qk_tile(0, it)
            qk_tile(2, it)
            if it == 0:
                qk_tile(1, 0)
                qk_tile(3, 0)
            for t16 in range(4 * it, 4 * it + 4):
                v_tile(t16)
            fu = (outproj_units(2 * (it - 1)) + outproj_units(2 * it - 1)
                  if it > 0 else [])
            c = attn(it, 0, fu[0:2], carry=c)
            c = attn(it, 1, fu[2:4], carry=c)
            if it > 0:
                qk_tile(1, it)
                qk_tile(3, it)
            c = attn(it, 2, fu[4:6], carry=c)
            c = attn(it, 3, fu[6:8], carry=c)
        c()
        outproj_tp(6)
        outproj_tp(7)

    nc.compile()
    return nc


def _get_nc():
    if "nc" not in _CACHE:
        _CACHE["nc"] = _build_nc()
    return _CACHE["nc"]


def _shard_inputs(x, W_attn, b_attn, W_proj, b_proj):
    import ml_dtypes
    f16 = np.float16
    e4 = ml_dtypes.float8_e4m3
    slopes = (1.0 / np.power(2.0, np.arange(1, H + 1))).astype(np.float32)
    iota = np.arange(T, dtype=np.float32)
    ak = np.stack([np.ones(T, np.float32), iota]).astype(f16)      # [2, T]
    pp, ff = np.meshgrid(np.arange(128), np.arange(128), indexing="ij")
    umin = np.where(pp <= ff, 30000.0, 0.0).astype(f16)
    # x transposed, quarter-major packed: [128, 4, 8, 512]
    xqs, x8s = [], []
    for b in range(x.shape[0]):
        xT = np.ascontiguousarray(x[b].T)                          # [C, T] f32
        xqs.append(np.ascontiguousarray(
            xT.reshape(8, 128, 4, IW).transpose(1, 2, 0, 3)).astype(f16))
        x8s.append(np.ascontiguousarray(
            np.clip(xT, -240, 240).reshape(4, 2, 128, 4, IW)
            .transpose(2, 3, 0, 1, 4)).astype(e4))

    in_maps = []
    for core in range(8):
        b, g = core // 4, core % 4
        heads = [g, g + 4, g + 8, g + 12]                          # slot 0 banded
        hcols = np.concatenate([np.arange(h * D, (h + 1) * D) for h in heads])
        q_cols = W_attn[:, 0:C][:, hcols] * 32.0    # 0.125 sm-scale * 256
        k_cols = W_attn[:, C:2 * C][:, hcols] * 32.0
        v_cols = W_attn[:, 2 * C:3 * C][:, hcols]
        wqk_l = np.concatenate([q_cols, k_cols], axis=1)           # [C, 512]
        wqk8_p = np.ascontiguousarray(
            np.clip(wqk_l, -240, 240).reshape(4, 2, 128, 4, 128)
            .transpose(2, 3, 0, 1, 4)).astype(e4)
        bq_l = (b_attn[0:C][hcols] * 0.125).astype(np.float32)     # [256]
        bq_p = np.ascontiguousarray(bq_l.reshape(2, 128).T)        # [128, 2]
        wv_p = np.ascontiguousarray(
            v_cols.reshape(8, 128, HD).transpose(1, 0, 2)).astype(f16)
        wp_l = W_proj[hcols, :]                                    # [256, C]
        wp_p = np.ascontiguousarray(
            wp_l.reshape(2, 128, C).transpose(1, 0, 2)).astype(f16)
        aq = np.zeros((2 * HL, T), np.float32)
        for hh in range(HL):
            s = slopes[heads[hh]]
            aq[2 * hh, :] = -s * iota
            aq[2 * hh + 1, :] = s
        in_maps.append({
            "xq": xqs[b], "x8": x8s[b], "wqk8": wqk8_p, "bq": bq_p,
            "wv": wv_p, "wp": wp_p,
            "aq": aq.astype(f16), "ak": ak, "umin": umin,
        })
    return in_maps


def kernel(x, W_attn, b_attn, W_proj, b_proj, _trace=False, _tmpdir=None):
    from concourse.bass_utils import run_bass_kernel_spmd

    x = np.asarray(x, dtype=np.float32)
    W_attn = np.asarray(W_attn, dtype=np.float32)
    b_attn = np.asarray(b_attn, dtype=np.float32)
    W_proj = np.asarray(W_proj, dtype=np.float32)
    b_proj = np.asarray(b_proj, dtype=np.float32)

    nc = _get_nc()
    in_maps = _shard_inputs(x, W_attn, b_attn, W_proj, b_proj)
    res = run_bass_kernel_spmd(
        nc, in_maps, core_ids=list(range(8)), trace=_trace, tmpdir=_tmpdir)
    # v-bias commutes through softmax: fold bv @ W_proj into the output bias.
    b_eff = b_proj + b_attn[2 * C:3 * C] @ W_proj
    out = np.empty((x.shape[0], T, C), np.float32)
    for b in range(x.shape[0]):
        acc = None
        for g in range(4):
            o = res.results[4 * b + g]["out"].astype(np.float32)   # [128,16,C]
            o = o.transpose(1, 0, 2).reshape(T, C)
            acc = o if acc is None else acc + o
        out[b] = acc + b_eff
    if _trace:
        kernel.last_exec_time_ns = res.exec_time_ns
    return out
